# revision 19
# baseline (speedup 1.0000x reference)
"""ChebConv (K=3) forward as a distributed Bass/Tile kernel on 8 trn2 NeuronCores.

Sharding (per spec hint): vertices V are sharded across the 8 cores.
  x0 = [x[0] | x[1]]            # [V, 128], feature col = b*64 + fin
  x1 = L @ x0                   # SpMM (COO, edge-parallel)
  x2' = L @ x1 - 0.5 x0         # = x2/2; the 2x is folded into W_k2
  out[b,v,:] = bias + sum_k xk[v, b*64:(b+1)*64] @ Wk'

Each core owns a row shard (V/8 rows padded to a multiple of 128 = "blocks").
SpMM per core and per 128-edge tile (all data bf16, PSUM accumulate f32):
  - gpsimd.dma_gather fetches the 256B source feature rows from per-piece
    tables (int16 indices), spread round-robin over 4 SWDGE queues.
  - The selector M[e,j] = val[e] * (lrow[e]==j) is built ON-CHIP per tile
    with one fused tensor_scalar (iota==lrow)*val from a 4B/edge
    (lrow,val) stream, alternating DVE / GpSimd engines.
  - PE matmul M^T @ G (spmm1, row-major out) or G^T @ M (spmm2, transposed
    out) performs the scaled segmented sum into a per-block PSUM accumulator.

The column space is split into npc "pieces" = block-ranges sliced ACROSS all
cores (piece j = blocks [B_j, B_j+1) of every core's shard, core-major).
Phase 1 computes x1 blocks in increasing order, and the AllGather of piece j
is issued as soon as its last block is stored, so the collective overlaps
phase 1 and phase 2's gathers (which consume pieces in arrival order) start
immediately. The final channel mix is fused into the SpMM2 block loop using
block-diagonal weights plus a rank-1 bias matmul; -0.5 x0 enters SpMM2's
accumulation as a (-0.5 I) matmul.

The tile structure is computed from the actual edge data at call time (max
over cores per (block, piece) slot) so one SPMD program fits all 8 cores.
"""

import sys

sys.path.insert(0, "/opt/trn_rl_repo")

import numpy as np
import ml_dtypes

import concourse.bass as bass
import concourse.bacc as bacc
import concourse.mybir as mybir
import concourse.tile as tile
from concourse import bass_utils
from concourse.alu_op_type import AluOpType

P = 128
F32 = mybir.dt.float32
BF16 = mybir.dt.bfloat16
I16 = mybir.dt.int16
NPBF16 = ml_dtypes.bfloat16
NQ = 4  # SWDGE queues (parallel descriptor feed; ucode max)


def _cdiv(a, b):
    return -(-a // b)


# ---------------------------------------------------------------------------
# Host-side: uniform (cross-core) edge structure + per-core content arrays
# ---------------------------------------------------------------------------


class EdgeStructure:
    def __init__(self, V, ncores, sb_blocks, npc, rows, cols):
        assert V % ncores == 0
        self.V, self.ncores = V, ncores
        self.vsh = V // ncores
        self.nblk = _cdiv(self.vsh, P)
        self.vpad = self.nblk * P
        self.vtot = self.vpad * ncores

        # pieces: block ranges of every core's shard, sliced across cores.
        # piece j table = [ncores * Rj * P rows]; must fit int16 indices.
        # Sizes tuned so per-(block,piece) edge counts land just UNDER a
        # multiple of 128 (less ceil-quantization padding).
        if self.nblk == 98 and npc == 4:
            Rj = [28, 28, 32, 10]
        else:
            base = self.nblk // npc
            rem = self.nblk - base * npc
            Rj = [base + (1 if j < rem else 0) for j in range(npc)]
        assert all(r * ncores * P <= 32768 for r in Rj)
        self.npc = npc
        self.piece_blocks = Rj
        self.piece_b0 = np.concatenate(([0], np.cumsum(Rj)))  # block bounds
        self.piece_rows = [r * P for r in Rj]

        rows = np.asarray(rows, np.int64)
        cols = np.asarray(cols, np.int64)
        c_of = cols // self.vsh
        l_of = cols - c_of * self.vsh
        cblk = l_of // P
        piece = np.searchsorted(self.piece_b0, cblk, side="right") - 1
        # index within piece table: c * Rj*P + (l - B_j*P)
        pr = np.array(self.piece_rows)[piece]
        self.idx_in_piece = c_of * pr + (l_of - self.piece_b0[piece] * P)
        assert self.idx_in_piece.max() < 32768

        r_core = rows // self.vsh
        r_loc = rows - r_core * self.vsh
        blk = r_loc // P
        nchunks = npc

        # slot order: for sb: for chunk(piece): for block in sb
        sb_arr = blk // sb_blocks
        bi_arr = blk % sb_blocks
        bh_arr = np.minimum(sb_blocks, self.nblk - sb_arr * sb_blocks)
        sid = sb_arr * sb_blocks * nchunks + piece * bh_arr + bi_arr

        self.sb_blocks = sb_blocks
        self.nsb = _cdiv(self.nblk, sb_blocks)
        order = []
        for sb in range(self.nsb):
            b0 = sb * sb_blocks
            bh = min(sb_blocks, self.nblk - b0)
            for ch in range(nchunks):
                for bi in range(bh):
                    order.append((b0 + bi, ch))
        self.nslots = len(order)
        self.slot_block = np.array([b for b, _ in order], np.int64)
        self.slot_chunk = np.array([c for _, c in order], np.int64)

        counts = np.zeros((ncores, self.nslots), np.int64)
        np.add.at(counts, (r_core, sid), 1)
        T = _cdiv(np.max(counts, axis=0), P)

        # every block needs >=1 tile so its PSUM accumulator gets written
        blk_tiles = np.zeros(self.nblk, np.int64)
        np.add.at(blk_tiles, self.slot_block, T)
        for b in np.nonzero(blk_tiles == 0)[0]:
            sb, bi = b // sb_blocks, b % sb_blocks
            bh = min(sb_blocks, self.nblk - sb * sb_blocks)
            T[sb * sb_blocks * nchunks + 0 * bh + bi] = 1

        self.T = T
        self.slot_tile_base = np.concatenate(([0], np.cumsum(T)))[:-1]
        self.ntiles = int(np.sum(T))
        self.sid_of_edge = sid
        self.r_core_of_edge = r_core
        self.lrow_of_edge = (r_loc % P).astype(np.int64)

        # (sb, chunk) -> contiguous tile run
        self.runs = []  # per sb: list of (tile_start, ntiles, chunk)
        s = 0
        for sb in range(self.nsb):
            b0 = sb * sb_blocks
            bh = min(sb_blocks, self.nblk - b0)
            sb_runs = []
            for ch in range(nchunks):
                t0 = int(self.slot_tile_base[s])
                ntr = int(np.sum(T[s : s + bh]))
                if ntr > 0:
                    sb_runs.append((t0, ntr, ch))
                s += bh
            self.runs.append(sb_runs)
        self.max_run_tiles = max(
            nt for sb_runs in self.runs for _, nt, _ in sb_runs
        )

        tile_block = np.empty(self.ntiles, np.int64)
        for s in range(self.nslots):
            t0, ntr = self.slot_tile_base[s], T[s]
            tile_block[t0 : t0 + ntr] = self.slot_block[s]
        self.tile_block = tile_block
        self.tile_start = np.zeros(self.ntiles, bool)
        self.tile_stop = np.zeros(self.ntiles, bool)
        first, last = {}, {}
        for t in range(self.ntiles):
            b = int(tile_block[t])
            if b not in first:
                first[b] = t
            last[b] = t
        for t in first.values():
            self.tile_start[t] = True
        for t in last.values():
            self.tile_stop[t] = True

        # last superblock index per piece (for AllGather issue points)
        self.piece_last_sb = [
            (int(self.piece_b0[j + 1]) - 1) // sb_blocks for j in range(npc)
        ]

    def run_list(self):
        out = []
        for sb_runs in self.runs:
            out.extend(sb_runs)
        return out

    def per_core_arrays(self, core, vals):
        """idx (int16 wrapped+replicated) and bf16 M tiles for one core."""
        sel = np.nonzero(self.r_core_of_edge == core)[0]
        sid = self.sid_of_edge[sel]
        o = np.argsort(sid, kind="stable")
        sel, sid = sel[o], sid[o]
        start = np.searchsorted(sid, np.arange(self.nslots))
        rank = np.arange(len(sid)) - start[sid]
        pos = self.slot_tile_base[sid] * P + rank
        n = self.ntiles * P
        idx = np.zeros(n, np.int16)
        idx[pos] = self.idx_in_piece[sel].astype(np.int16)
        idx_w = np.tile(np.ascontiguousarray(idx.reshape(-1, 16).T), (8, 1))
        m = np.zeros((self.ntiles, P, P), np.float32)
        m[pos // P, pos % P, self.lrow_of_edge[sel]] = vals[sel]
        mfull = np.ascontiguousarray(
            m.astype(NPBF16).transpose(1, 0, 2).reshape(P, self.ntiles * P)
        )
        return idx_w, mfull


# ---------------------------------------------------------------------------
# Bass program (SPMD: one program, per-core data via in_maps)
# ---------------------------------------------------------------------------


def build_program(es: EdgeStructure):
    nblk, vpad, ncores = es.nblk, es.vpad, es.ncores
    nt, GW, SB, npc = es.ntiles, es.max_run_tiles, es.sb_blocks, es.npc

    nc = bacc.Bacc(
        "TRN2",
        target_bir_lowering=False,
        debug=False,
        num_devices=ncores,
        num_swdge_queues=NQ,
    )

    x0p = [
        nc.dram_tensor(f"x0p{j}", [ncores * es.piece_rows[j], P], BF16,
                       kind="ExternalInput")
        for j in range(npc)
    ]
    x0t = nc.dram_tensor("x0t", [nblk, P, P], BF16, kind="ExternalInput")
    wbd = nc.dram_tensor("wbd", [3, P, P], BF16, kind="ExternalInput")
    ident_d = nc.dram_tensor("ident", [P, P], BF16, kind="ExternalInput")
    eidx = nc.dram_tensor("eidx", [P, nt * 8], I16, kind="ExternalInput")
    emt = nc.dram_tensor("emt", [P, nt * P], BF16, kind="ExternalInput")
    outp = nc.dram_tensor("outp", [vpad, P], BF16, kind="ExternalOutput")

    x1my = [
        nc.dram_tensor(f"x1my{j}", [es.piece_rows[j], P], BF16)
        for j in range(npc)
    ]
    x1p = [
        nc.dram_tensor(f"x1p{j}", [ncores * es.piece_rows[j], P], BF16,
                       addr_space="Shared")
        for j in range(npc)
    ]

    with tile.TileContext(nc) as tc:
        with (
            tc.tile_pool(name="const", bufs=1) as cpool,
            tc.tile_pool(name="gslab", bufs=16) as gpool,
            tc.tile_pool(name="mslab", bufs=12) as mpool,
            tc.tile_pool(name="xio", bufs=4) as xpool,
            tc.tile_pool(name="ostage", bufs=6) as opool,
            tc.tile_pool(name="acc", bufs=2 * SB, space="PSUM") as apool,
            tc.tile_pool(name="ptr", bufs=1, space="PSUM") as ptpool,
            tc.tile_pool(name="pmix", bufs=1, space="PSUM") as pmpool,
        ):
            ident_s = cpool.tile([P, P], BF16, tag="ident")
            nc.sync.dma_start(out=ident_s[:], in_=ident_d[:, :])
            wbd_s = cpool.tile([P, 3 * P], BF16, tag="wbd")
            for k in range(3):
                nc.sync.dma_start(
                    out=wbd_s[:, k * P : (k + 1) * P], in_=wbd[k, :, :]
                )
            x1t_s = cpool.tile([P, nblk * P], BF16, tag="x1t")
            eidx_s = cpool.tile([P, nt * 8], I16, tag="eidx")
            nc.sync.dma_start(out=eidx_s[:], in_=eidx[:, :])

            qn = [0]

            def spmm(src_tabs, layout_b, out_cb, after_run=None,
                     pre_sb=None):
                for sb in range(es.nsb):
                    b0 = sb * SB
                    bh = min(SB, nblk - b0)
                    pre = pre_sb(sb, b0, bh) if pre_sb is not None else None
                    psums = {
                        b0 + bi: apool.tile(
                            [P, P], F32, tag="acc", name=f"acc{b0 + bi}"
                        )
                        for bi in range(bh)
                    }
                    for ri, (t0, ntr, ch) in enumerate(es.runs[sb]):
                        mt = mpool.tile([P, GW * P], BF16, tag="m")
                        nc.sync.dma_start(
                            out=mt[:, : ntr * P],
                            in_=emt[:, t0 * P : (t0 + ntr) * P],
                        )
                        g = gpool.tile([P, GW * P], BF16, tag="g")
                        nidx = ntr * P
                        nc.gpsimd.dma_gather(
                            out_ap=g[:, :nidx].rearrange(
                                "p (t e) -> p t e", e=P
                            ),
                            in_ap=src_tabs[ch][:, :],
                            idxs_ap=eidx_s[:, 8 * t0 : 8 * (t0 + ntr)],
                            num_idxs=nidx,
                            num_idxs_reg=nidx,
                            elem_size=P,
                            single_packet=False,
                            queue_num=qn[0] % NQ,
                        )
                        qn[0] += 1
                        for tt in range(ntr):
                            t = t0 + tt
                            b = int(es.tile_block[t])
                            gt = g[:, tt * P : (tt + 1) * P]
                            mm = mt[:, tt * P : (tt + 1) * P]
                            start = bool(es.tile_start[t])
                            stop = bool(es.tile_stop[t])
                            if layout_b:
                                nc.tensor.matmul(
                                    out=psums[b][:], lhsT=gt, rhs=mm,
                                    start=start, stop=stop,
                                )
                            else:
                                nc.tensor.matmul(
                                    out=psums[b][:], lhsT=mm, rhs=gt,
                                    start=start, stop=stop,
                                )
                        if after_run is not None:
                            after_run(sb, ri)
                    for bi in range(bh):
                        out_cb(b0 + bi, psums[b0 + bi], pre)

            # ---------------- SpMM 1: x1 = L @ x0 (row-major out) --------
            def cb1(b, ps, _pre):
                xb = opool.tile([P, P], BF16, tag="x1st")
                nc.vector.tensor_copy(xb[:], ps[:])
                j = int(np.searchsorted(es.piece_b0, b, side="right")) - 1
                r0 = (b - int(es.piece_b0[j])) * P
                nc.scalar.dma_start(
                    out=x1my[j][r0 : r0 + P, :], in_=xb[:]
                )
                pt = ptpool.tile([P, P], BF16, tag="ptr")
                nc.tensor.transpose(
                    out=pt[:], in_=xb[:], identity=ident_s[:]
                )
                nc.vector.tensor_copy(x1t_s[:, b * P : (b + 1) * P], pt[:])

            def issue_ag(j):
                nc.gpsimd.collective_compute(
                    "AllGather",
                    AluOpType.bypass,
                    replica_groups=[list(range(ncores))],
                    ins=[x1my[j].ap().opt()],
                    outs=[x1p[j].ap().opt()],
                )

            # issue piece-j AllGather one superblock AFTER its last store,
            # so its deps are already resolved when the Pool queue reaches
            # it (a parked collective head-of-line blocks later gathers).
            def after_run1(sb, ri):
                if ri == 0:
                    for j in range(npc - 1):
                        if es.piece_last_sb[j] + 1 == sb:
                            issue_ag(j)

            spmm(x0p, False, cb1, after_run=after_run1)

            # -------- SpMM 2 (transposed out) + fused channel mix --------
            def pre_sb2(sb, b0, bh):
                xsb = xpool.tile([P, SB * P], BF16, tag="x0sb")
                nc.sync.dma_start(
                    out=xsb[:, : bh * P].rearrange("p (b f) -> p b f", f=P),
                    in_=x0t[b0 : b0 + bh, :, :].rearrange("b p f -> p b f"),
                )
                return xsb

            def cb2(b, ps, xsb):
                # ps = (L x1)^T block; the -0.5 x0 term is folded into wbd_0
                bi = b % SB
                x0b = xsb[:, bi * P : (bi + 1) * P]
                x2b = opool.tile([P, P], BF16, tag="x2b")
                nc.vector.tensor_copy(x2b[:], ps[:])
                x1tb = x1t_s[:, b * P : (b + 1) * P]
                # channel mix: out = sum_k XkT^T @ Wbd_k (bias on host)
                pm = pmpool.tile([P, P], F32, tag="pmix")
                for k, xk in enumerate((x0b, x1tb, x2b)):
                    nc.tensor.matmul(
                        out=pm[:],
                        lhsT=xk if k != 2 else xk[:],
                        rhs=wbd_s[:, k * P : (k + 1) * P],
                        start=(k == 0),
                        stop=(k == 2),
                    )
                ob = opool.tile([P, P], BF16, tag="ob")
                nc.vector.tensor_copy(ob[:], pm[:])
                nc.scalar.dma_start(
                    out=outp[b * P : (b + 1) * P, :], in_=ob[:]
                )

            # The last piece's AllGather is issued inside phase 2, after
            # sb0's ch0-2 runs (which don't need it) — so phase-2 gathers
            # start immediately at the phase boundary instead of queueing
            # behind a parked collective on the Pool engine.
            def after_run2(sb, ri):
                if sb == 0 and ri == 2:
                    issue_ag(npc - 1)

            spmm(x1p, True, cb2, after_run=after_run2, pre_sb=pre_sb2)

    nc.compile()
    return nc


# ---------------------------------------------------------------------------
# Host driver
# ---------------------------------------------------------------------------


def prepare(x, weight, bias, lap_vals, lap_rows, lap_cols, ncores=8,
            sb_blocks=3, npc=4):
    x = np.asarray(x, np.float32)
    weight = np.asarray(weight, np.float32)
    bias = np.asarray(bias, np.float32)
    lap_vals = np.asarray(lap_vals, np.float32)
    lap_rows = np.asarray(lap_rows)
    lap_cols = np.asarray(lap_cols)
    B, V, FIN = x.shape
    _, K, FOUT = weight.shape
    assert B == 2 and FIN == 64 and K == 3 and FOUT == 64

    es = EdgeStructure(V, ncores, sb_blocks, npc, lap_rows, lap_cols)

    x0 = np.concatenate([x[0], x[1]], axis=1)  # [V, 128] f32
    # padded per-core [vpad, P] bf16 shards
    xsh = np.zeros((ncores, es.vpad, P), NPBF16)
    for c in range(ncores):
        xsh[c, : es.vsh] = x0[c * es.vsh : (c + 1) * es.vsh].astype(NPBF16)
    # per-piece tables [ncores * Rj*P, P]
    x0p_arrs = []
    for j in range(npc):
        lo, hi = int(es.piece_b0[j]) * P, int(es.piece_b0[j + 1]) * P
        x0p_arrs.append(
            np.ascontiguousarray(xsh[:, lo:hi].reshape(-1, P))
        )

    wbd = np.zeros((3, P, P), np.float32)
    for k in range(3):
        wk = weight[:, k, :] * (2.0 if k == 2 else 1.0)  # x2' = x2/2
        if k == 0:
            wk = wk - weight[:, 2, :]  # fold -0.5 x0 of x2' into x0 term
        wbd[k, :64, :64] = wk
        wbd[k, 64:, 64:] = wk
    wbd = wbd.astype(NPBF16)
    ident = np.eye(P, dtype=np.float32).astype(NPBF16)
    in_maps = []
    for c in range(ncores):
        idx_w, mfull = es.per_core_arrays(c, lap_vals)
        x0t_c = np.ascontiguousarray(
            xsh[c].reshape(es.nblk, P, P).transpose(0, 2, 1)
        )
        im = {
            "x0t": x0t_c,
            "wbd": wbd,
            "ident": ident,
            "eidx": idx_w,
            "emt": mfull,
        }
        for j in range(npc):
            im[f"x0p{j}"] = x0p_arrs[j]
        in_maps.append(im)

    nc = build_program(es)

    def assemble(results):
        out = np.empty((B, V, FOUT), np.float32)
        for c in range(ncores):
            o = np.asarray(results[c]["outp"]).astype(np.float32)
            out[0, c * es.vsh : (c + 1) * es.vsh, :] = o[: es.vsh, :64]
            out[1, c * es.vsh : (c + 1) * es.vsh, :] = o[: es.vsh, 64:]
        return out + bias[None, None, :]

    return nc, in_maps, assemble, es


def kernel(x, weight, bias, lap_vals, lap_rows, lap_cols):
    nc, in_maps, assemble, es = prepare(
        x, weight, bias, lap_vals, lap_rows, lap_cols
    )
    res = bass_utils.run_bass_kernel_spmd(
        nc, in_maps, core_ids=list(range(es.ncores))
    )
    return assemble(res.results)


# revision 21
# speedup vs baseline: 1.1628x; 1.1628x over previous
"""ChebConv (K=3) forward as a distributed Bass/Tile kernel on 8 trn2 NeuronCores.

Sharding (per spec hint): vertices V are sharded across the 8 cores.
  x0 = [x[0] | x[1]]            # [V, 128], feature col = b*64 + fin
  x1 = L @ x0                   # SpMM (COO, edge-parallel)
  x2' = L @ x1 - 0.5 x0         # = x2/2; the 2x is folded into W_k2
  out[b,v,:] = bias + sum_k xk[v, b*64:(b+1)*64] @ Wk'

Each core owns a row shard (V/8 rows padded to a multiple of 128 = "blocks").
SpMM per core and per 128-edge tile (all data bf16, PSUM accumulate f32):
  - gpsimd.dma_gather fetches the 256B source feature rows from per-piece
    tables (int16 indices), spread round-robin over 4 SWDGE queues.
  - The selector M[e,j] = val[e] * (lrow[e]==j) is built ON-CHIP per tile
    with one fused tensor_scalar (iota==lrow)*val from a 4B/edge
    (lrow,val) stream, alternating DVE / GpSimd engines.
  - PE matmul M^T @ G (spmm1, row-major out) or G^T @ M (spmm2, transposed
    out) performs the scaled segmented sum into a per-block PSUM accumulator.

The column space is split into npc "pieces" = block-ranges sliced ACROSS all
cores (piece j = blocks [B_j, B_j+1) of every core's shard, core-major).
Phase 1 computes x1 blocks in increasing order, and the AllGather of piece j
is issued as soon as its last block is stored, so the collective overlaps
phase 1 and phase 2's gathers (which consume pieces in arrival order) start
immediately. The final channel mix is fused into the SpMM2 block loop using
block-diagonal weights plus a rank-1 bias matmul; -0.5 x0 enters SpMM2's
accumulation as a (-0.5 I) matmul.

The tile structure is computed from the actual edge data at call time (max
over cores per (block, piece) slot) so one SPMD program fits all 8 cores.
"""

import sys

sys.path.insert(0, "/opt/trn_rl_repo")

import numpy as np
import ml_dtypes

import concourse.bass as bass
import concourse.bacc as bacc
import concourse.mybir as mybir
import concourse.tile as tile
from concourse import bass_utils
from concourse.alu_op_type import AluOpType

P = 128
F32 = mybir.dt.float32
BF16 = mybir.dt.bfloat16
I16 = mybir.dt.int16
NPBF16 = ml_dtypes.bfloat16
NQ = 4  # SWDGE queues (parallel descriptor feed; ucode max)


def _cdiv(a, b):
    return -(-a // b)


# ---------------------------------------------------------------------------
# Host-side: uniform (cross-core) edge structure + per-core content arrays
# ---------------------------------------------------------------------------


class EdgeStructure:
    def __init__(self, V, ncores, sb_blocks, npc, rows, cols):
        assert V % ncores == 0
        self.V, self.ncores = V, ncores
        self.vsh = V // ncores
        self.nblk = _cdiv(self.vsh, P)
        self.vpad = self.nblk * P
        self.vtot = self.vpad * ncores

        # pieces: block ranges of every core's shard, sliced across cores.
        # piece j table = [ncores * Rj * P rows]; must fit int16 indices.
        # Sizes tuned so per-(block,piece) edge counts land just UNDER a
        # multiple of 128 (less ceil-quantization padding).
        if self.nblk == 98 and npc == 4:
            Rj = [28, 28, 21, 21]
        else:
            base = self.nblk // npc
            rem = self.nblk - base * npc
            Rj = [base + (1 if j < rem else 0) for j in range(npc)]
        assert all(r * ncores * P <= 32768 for r in Rj)
        self.npc = npc
        self.piece_blocks = Rj
        self.piece_b0 = np.concatenate(([0], np.cumsum(Rj)))  # block bounds
        self.piece_rows = [r * P for r in Rj]

        rows = np.asarray(rows, np.int64)
        cols = np.asarray(cols, np.int64)
        c_of = cols // self.vsh
        l_of = cols - c_of * self.vsh
        cblk = l_of // P
        piece = np.searchsorted(self.piece_b0, cblk, side="right") - 1
        # index within piece table: c * Rj*P + (l - B_j*P)
        pr = np.array(self.piece_rows)[piece]
        self.idx_in_piece = c_of * pr + (l_of - self.piece_b0[piece] * P)
        assert self.idx_in_piece.max() < 32768

        r_core = rows // self.vsh
        r_loc = rows - r_core * self.vsh
        blk = r_loc // P
        nchunks = npc

        # slot order: for sb: for chunk(piece): for block in sb
        sb_arr = blk // sb_blocks
        bi_arr = blk % sb_blocks
        bh_arr = np.minimum(sb_blocks, self.nblk - sb_arr * sb_blocks)
        sid = sb_arr * sb_blocks * nchunks + piece * bh_arr + bi_arr

        self.sb_blocks = sb_blocks
        self.nsb = _cdiv(self.nblk, sb_blocks)
        order = []
        for sb in range(self.nsb):
            b0 = sb * sb_blocks
            bh = min(sb_blocks, self.nblk - b0)
            for ch in range(nchunks):
                for bi in range(bh):
                    order.append((b0 + bi, ch))
        self.nslots = len(order)
        self.slot_block = np.array([b for b, _ in order], np.int64)
        self.slot_chunk = np.array([c for _, c in order], np.int64)

        counts = np.zeros((ncores, self.nslots), np.int64)
        np.add.at(counts, (r_core, sid), 1)
        T = _cdiv(np.max(counts, axis=0), P)

        # every block needs >=1 tile so its PSUM accumulator gets written
        blk_tiles = np.zeros(self.nblk, np.int64)
        np.add.at(blk_tiles, self.slot_block, T)
        for b in np.nonzero(blk_tiles == 0)[0]:
            sb, bi = b // sb_blocks, b % sb_blocks
            bh = min(sb_blocks, self.nblk - sb * sb_blocks)
            T[sb * sb_blocks * nchunks + 0 * bh + bi] = 1

        self.T = T
        self.slot_tile_base = np.concatenate(([0], np.cumsum(T)))[:-1]
        self.ntiles = int(np.sum(T))
        self.sid_of_edge = sid
        self.r_core_of_edge = r_core
        self.lrow_of_edge = (r_loc % P).astype(np.int64)

        # (sb, chunk) -> contiguous tile run
        self.runs = []  # per sb: list of (tile_start, ntiles, chunk)
        s = 0
        for sb in range(self.nsb):
            b0 = sb * sb_blocks
            bh = min(sb_blocks, self.nblk - b0)
            sb_runs = []
            for ch in range(nchunks):
                t0 = int(self.slot_tile_base[s])
                ntr = int(np.sum(T[s : s + bh]))
                if ntr > 0:
                    sb_runs.append((t0, ntr, ch))
                s += bh
            self.runs.append(sb_runs)
        self.max_run_tiles = max(
            nt for sb_runs in self.runs for _, nt, _ in sb_runs
        )

        tile_block = np.empty(self.ntiles, np.int64)
        for s in range(self.nslots):
            t0, ntr = self.slot_tile_base[s], T[s]
            tile_block[t0 : t0 + ntr] = self.slot_block[s]
        self.tile_block = tile_block
        self.tile_start = np.zeros(self.ntiles, bool)
        self.tile_stop = np.zeros(self.ntiles, bool)
        first, last = {}, {}
        for t in range(self.ntiles):
            b = int(tile_block[t])
            if b not in first:
                first[b] = t
            last[b] = t
        for t in first.values():
            self.tile_start[t] = True
        for t in last.values():
            self.tile_stop[t] = True

        # last superblock index per piece (for AllGather issue points)
        self.piece_last_sb = [
            (int(self.piece_b0[j + 1]) - 1) // sb_blocks for j in range(npc)
        ]

    def run_list(self):
        out = []
        for sb_runs in self.runs:
            out.extend(sb_runs)
        return out

    def per_core_arrays(self, core, vals):
        """idx (int16 wrapped+replicated) and bf16 M tiles for one core."""
        sel = np.nonzero(self.r_core_of_edge == core)[0]
        sid = self.sid_of_edge[sel]
        o = np.argsort(sid, kind="stable")
        sel, sid = sel[o], sid[o]
        start = np.searchsorted(sid, np.arange(self.nslots))
        rank = np.arange(len(sid)) - start[sid]
        pos = self.slot_tile_base[sid] * P + rank
        n = self.ntiles * P
        idx = np.zeros(n, np.int16)
        idx[pos] = self.idx_in_piece[sel].astype(np.int16)
        idx_w = np.tile(np.ascontiguousarray(idx.reshape(-1, 16).T), (8, 1))
        m = np.zeros((self.ntiles, P, P), np.float32)
        m[pos // P, pos % P, self.lrow_of_edge[sel]] = vals[sel]
        mfull = np.ascontiguousarray(
            m.astype(NPBF16).transpose(1, 0, 2).reshape(P, self.ntiles * P)
        )
        return idx_w, mfull


# ---------------------------------------------------------------------------
# Bass program (SPMD: one program, per-core data via in_maps)
# ---------------------------------------------------------------------------


def build_program(es: EdgeStructure):
    nblk, vpad, ncores = es.nblk, es.vpad, es.ncores
    nt, GW, SB, npc = es.ntiles, es.max_run_tiles, es.sb_blocks, es.npc

    nc = bacc.Bacc(
        "TRN2",
        target_bir_lowering=False,
        debug=False,
        num_devices=ncores,
        num_swdge_queues=NQ,
    )

    x0p = [
        nc.dram_tensor(f"x0p{j}", [ncores * es.piece_rows[j], P], BF16,
                       kind="ExternalInput")
        for j in range(npc)
    ]
    x0t = nc.dram_tensor("x0t", [nblk, P, P], BF16, kind="ExternalInput")
    wbd = nc.dram_tensor("wbd", [3, P, P], BF16, kind="ExternalInput")
    ident_d = nc.dram_tensor("ident", [P, P], BF16, kind="ExternalInput")
    eidx = nc.dram_tensor("eidx", [P, nt * 8], I16, kind="ExternalInput")
    emt = nc.dram_tensor("emt", [P, nt * P], BF16, kind="ExternalInput")
    outp = nc.dram_tensor("outp", [vpad, P], BF16, kind="ExternalOutput")

    x1my = [
        nc.dram_tensor(f"x1my{j}", [es.piece_rows[j], P], BF16)
        for j in range(npc)
    ]
    x1p = [
        nc.dram_tensor(f"x1p{j}", [ncores * es.piece_rows[j], P], BF16)
        for j in range(npc)
    ]

    with tile.TileContext(nc) as tc:
        with (
            tc.tile_pool(name="const", bufs=1) as cpool,
            tc.tile_pool(name="gslab", bufs=16) as gpool,
            tc.tile_pool(name="mslab", bufs=12) as mpool,
            tc.tile_pool(name="xio", bufs=4) as xpool,
            tc.tile_pool(name="ostage", bufs=6) as opool,
            tc.tile_pool(name="acc", bufs=2 * SB, space="PSUM") as apool,
            tc.tile_pool(name="ptr", bufs=1, space="PSUM") as ptpool,
            tc.tile_pool(name="pmix", bufs=1, space="PSUM") as pmpool,
        ):
            ident_s = cpool.tile([P, P], BF16, tag="ident")
            nc.sync.dma_start(out=ident_s[:], in_=ident_d[:, :])
            wbd_s = cpool.tile([P, 3 * P], BF16, tag="wbd")
            for k in range(3):
                nc.sync.dma_start(
                    out=wbd_s[:, k * P : (k + 1) * P], in_=wbd[k, :, :]
                )
            x1t_s = cpool.tile([P, nblk * P], BF16, tag="x1t")
            eidx_s = cpool.tile([P, nt * 8], I16, tag="eidx")
            nc.sync.dma_start(out=eidx_s[:], in_=eidx[:, :])

            qn = [0]

            def spmm(src_tabs, layout_b, out_cb, after_run=None,
                     pre_sb=None):
                for sb in range(es.nsb):
                    b0 = sb * SB
                    bh = min(SB, nblk - b0)
                    pre = pre_sb(sb, b0, bh) if pre_sb is not None else None
                    psums = {
                        b0 + bi: apool.tile(
                            [P, P], F32, tag="acc", name=f"acc{b0 + bi}"
                        )
                        for bi in range(bh)
                    }
                    for ri, (t0, ntr, ch) in enumerate(es.runs[sb]):
                        mt = mpool.tile([P, GW * P], BF16, tag="m")
                        nc.sync.dma_start(
                            out=mt[:, : ntr * P],
                            in_=emt[:, t0 * P : (t0 + ntr) * P],
                        )
                        g = gpool.tile([P, GW * P], BF16, tag="g")
                        nidx = ntr * P
                        nc.gpsimd.dma_gather(
                            out_ap=g[:, :nidx].rearrange(
                                "p (t e) -> p t e", e=P
                            ),
                            in_ap=src_tabs[ch][:, :],
                            idxs_ap=eidx_s[:, 8 * t0 : 8 * (t0 + ntr)],
                            num_idxs=nidx,
                            num_idxs_reg=nidx,
                            elem_size=P,
                            single_packet=False,
                            queue_num=qn[0] % NQ,
                        )
                        qn[0] += 1
                        for tt in range(ntr):
                            t = t0 + tt
                            b = int(es.tile_block[t])
                            gt = g[:, tt * P : (tt + 1) * P]
                            mm = mt[:, tt * P : (tt + 1) * P]
                            start = bool(es.tile_start[t])
                            stop = bool(es.tile_stop[t])
                            if layout_b:
                                nc.tensor.matmul(
                                    out=psums[b][:], lhsT=gt, rhs=mm,
                                    start=start, stop=stop,
                                )
                            else:
                                nc.tensor.matmul(
                                    out=psums[b][:], lhsT=mm, rhs=gt,
                                    start=start, stop=stop,
                                )
                        if after_run is not None:
                            after_run(sb, ri)
                    for bi in range(bh):
                        out_cb(b0 + bi, psums[b0 + bi], pre)

            # ---------------- SpMM 1: x1 = L @ x0 (row-major out) --------
            def cb1(b, ps, _pre):
                xb = opool.tile([P, P], BF16, tag="x1st")
                nc.vector.tensor_copy(xb[:], ps[:])
                j = int(np.searchsorted(es.piece_b0, b, side="right")) - 1
                r0 = (b - int(es.piece_b0[j])) * P
                nc.scalar.dma_start(
                    out=x1my[j][r0 : r0 + P, :], in_=xb[:]
                )
                pt = ptpool.tile([P, P], BF16, tag="ptr")
                nc.tensor.transpose(
                    out=pt[:], in_=xb[:], identity=ident_s[:]
                )
                nc.vector.tensor_copy(x1t_s[:, b * P : (b + 1) * P], pt[:])

            def issue_ag(j):
                nc.gpsimd.collective_compute(
                    "AllGather",
                    AluOpType.bypass,
                    replica_groups=[list(range(ncores))],
                    ins=[x1my[j].ap().opt()],
                    outs=[x1p[j].ap().opt()],
                )

            # issue piece-j AllGather one superblock AFTER its last store,
            # so its deps are already resolved when the Pool queue reaches
            # it (a parked collective head-of-line blocks later gathers).
            def after_run1(sb, ri):
                if ri == 0:
                    for j in range(npc - 1):
                        if es.piece_last_sb[j] + 1 == sb:
                            issue_ag(j)

            spmm(x0p, False, cb1, after_run=after_run1)
            issue_ag(npc - 1)

            # -------- SpMM 2 (transposed out) + fused channel mix --------
            def pre_sb2(sb, b0, bh):
                xsb = xpool.tile([P, SB * P], BF16, tag="x0sb")
                nc.sync.dma_start(
                    out=xsb[:, : bh * P].rearrange("p (b f) -> p b f", f=P),
                    in_=x0t[b0 : b0 + bh, :, :].rearrange("b p f -> p b f"),
                )
                return xsb

            def cb2(b, ps, xsb):
                # ps = (L x1)^T block; the -0.5 x0 term is folded into wbd_0
                bi = b % SB
                x0b = xsb[:, bi * P : (bi + 1) * P]
                x2b = opool.tile([P, P], BF16, tag="x2b")
                nc.vector.tensor_copy(x2b[:], ps[:])
                x1tb = x1t_s[:, b * P : (b + 1) * P]
                # channel mix: out = sum_k XkT^T @ Wbd_k (bias on host)
                pm = pmpool.tile([P, P], F32, tag="pmix")
                for k, xk in enumerate((x0b, x1tb, x2b)):
                    nc.tensor.matmul(
                        out=pm[:],
                        lhsT=xk if k != 2 else xk[:],
                        rhs=wbd_s[:, k * P : (k + 1) * P],
                        start=(k == 0),
                        stop=(k == 2),
                    )
                ob = opool.tile([P, P], BF16, tag="ob")
                nc.vector.tensor_copy(ob[:], pm[:])
                nc.scalar.dma_start(
                    out=outp[b * P : (b + 1) * P, :], in_=ob[:]
                )

            # AG for the last piece was issued at the end of phase 1;
            # phase-2 runs consume pieces in order so sb0/ch3 may briefly
            # wait on it, later sbs never do.
            spmm(x1p, True, cb2, pre_sb=pre_sb2)

    nc.compile()
    return nc


# ---------------------------------------------------------------------------
# Host driver
# ---------------------------------------------------------------------------


def prepare(x, weight, bias, lap_vals, lap_rows, lap_cols, ncores=8,
            sb_blocks=3, npc=4):
    x = np.asarray(x, np.float32)
    weight = np.asarray(weight, np.float32)
    bias = np.asarray(bias, np.float32)
    lap_vals = np.asarray(lap_vals, np.float32)
    lap_rows = np.asarray(lap_rows)
    lap_cols = np.asarray(lap_cols)
    B, V, FIN = x.shape
    _, K, FOUT = weight.shape
    assert B == 2 and FIN == 64 and K == 3 and FOUT == 64

    es = EdgeStructure(V, ncores, sb_blocks, npc, lap_rows, lap_cols)

    x0 = np.concatenate([x[0], x[1]], axis=1)  # [V, 128] f32
    # padded per-core [vpad, P] bf16 shards
    xsh = np.zeros((ncores, es.vpad, P), NPBF16)
    for c in range(ncores):
        xsh[c, : es.vsh] = x0[c * es.vsh : (c + 1) * es.vsh].astype(NPBF16)
    # per-piece tables [ncores * Rj*P, P]
    x0p_arrs = []
    for j in range(npc):
        lo, hi = int(es.piece_b0[j]) * P, int(es.piece_b0[j + 1]) * P
        x0p_arrs.append(
            np.ascontiguousarray(xsh[:, lo:hi].reshape(-1, P))
        )

    wbd = np.zeros((3, P, P), np.float32)
    for k in range(3):
        wk = weight[:, k, :] * (2.0 if k == 2 else 1.0)  # x2' = x2/2
        if k == 0:
            wk = wk - weight[:, 2, :]  # fold -0.5 x0 of x2' into x0 term
        wbd[k, :64, :64] = wk
        wbd[k, 64:, 64:] = wk
    wbd = wbd.astype(NPBF16)
    ident = np.eye(P, dtype=np.float32).astype(NPBF16)
    in_maps = []
    for c in range(ncores):
        idx_w, mfull = es.per_core_arrays(c, lap_vals)
        x0t_c = np.ascontiguousarray(
            xsh[c].reshape(es.nblk, P, P).transpose(0, 2, 1)
        )
        im = {
            "x0t": x0t_c,
            "wbd": wbd,
            "ident": ident,
            "eidx": idx_w,
            "emt": mfull,
        }
        for j in range(npc):
            im[f"x0p{j}"] = x0p_arrs[j]
        in_maps.append(im)

    nc = build_program(es)

    def assemble(results):
        out = np.empty((B, V, FOUT), np.float32)
        for c in range(ncores):
            o = np.asarray(results[c]["outp"]).astype(np.float32)
            out[0, c * es.vsh : (c + 1) * es.vsh, :] = o[: es.vsh, :64]
            out[1, c * es.vsh : (c + 1) * es.vsh, :] = o[: es.vsh, 64:]
        return out + bias[None, None, :]

    return nc, in_maps, assemble, es


def kernel(x, weight, bias, lap_vals, lap_rows, lap_cols):
    nc, in_maps, assemble, es = prepare(
        x, weight, bias, lap_vals, lap_rows, lap_cols
    )
    res = bass_utils.run_bass_kernel_spmd(
        nc, in_maps, core_ids=list(range(es.ncores))
    )
    return assemble(res.results)
